# revision 13
# baseline (speedup 1.0000x reference)
"""Trainium2 Bass kernel for EnhancedInvariantExtractor (v2).

Input  h [1_000_000, 120] f32:  per atom: 32 scalars | 16 vectors (l=1, dim 3)
                                | 8 tensors (l=2, dim 5).
Output [1_000_000, 204] f32: scalars(32) | vnorm(16) | tnorm(8) | vdots(120)
                             | tdots(28) (clipped pairwise cosines, upper
                             triangle, row-major).

Strategy (8 NeuronCores, data-parallel over atoms). v2 changes vs baseline:
- fp16 I/O: host ships hT [88, PADDED] f16 (vec+tens components only; the
  32 scalar columns are a pure identity passthrough done on host).
  Device outputs dots [148, PADDED] f16 + norms [24, PADDED] f16.
- Norm path is ONE ACT op: rinv = Rsqrt(n2 + eps) — the
  'reciprocal_sqrt_and_small' act table holds Rsqrt+Square+Copy, so no
  act-table churn. norm = n2 * rinv on DVE (grouped over 4 chunks).
- Pair-sum chunking repacked 5 -> 4 PE passes: [40 vec pairs + 1 tens pair]
  x3 + [25 tens pairs], 125 rows each. Dots: 4 passes into 2 PSUM banks at
  row offsets 0/64 (R stationaries padded to 64 cols with zeros so every
  PSUM row is written).
- PE work per chunk of 512 atoms: n2, rexp, u x4, d x4 = 10 passes of 512
  cols. Emission order software-pipelines across 4-chunk groups (d-passes
  of older chunks fill the rsqrt/vu latency windows) to keep the tensor
  engine gapless: TRN2 ramps 1.2 -> 2.4 GHz only after ~3us of continuous
  PE execution.
- Squares u^2 and the PSUM->SBUF dots moves (0.5*d - 1 affine folded in)
  are spread across ACT / DVE / Pool so no single elementwise engine
  exceeds the PE wall.
"""

import sys

sys.path.insert(0, "/opt/trn_rl_repo")

import numpy as np

N_ATOMS = 1_000_000
N_CORES = 8
PER_CORE = N_ATOMS // N_CORES  # 125_000
CHUNK = 512
N_CHUNKS = 248  # 62 groups of 4
PADDED = CHUNK * N_CHUNKS  # 126_976
N_GROUPS = N_CHUNKS // 4
NV, NT = 16, 8
NRIN = 88  # device input rows: vec comps 48 | tens comps 40
NPAIR = 148
NOUT = 204
EPS2 = 1e-12
# pair chunking: 3 chunks of [40 v-pairs + 1 t-pair] + 1 chunk of [25 t-pairs]
PK = [41, 41, 41, 25]
RK = 125

_CACHE = {}


def _pairs():
    pv = [(i, j) for i in range(NV) for j in range(i + 1, NV)]
    pt = [(a, b) for a in range(NT) for b in range(a + 1, NT)]
    return pv, pt


def _chunk_pairs():
    """Four u-chunks; each entry is a list of (rows_i, rows_j) component-row
    tuples. Chunks 0-2: v-pairs 40k..40k+40 then t-pair k. Chunk 3:
    t-pairs 3..28."""
    pv, pt = _pairs()
    vrow = lambda i, d: 3 * i + d
    trow = lambda t, d: 48 + 5 * t + d
    chunks = []
    for k in range(3):
        ch = []
        for i, j in pv[40 * k : 40 * k + 40]:
            ch.append([(vrow(i, d), vrow(j, d)) for d in range(3)])
        a, b = pt[k]
        ch.append([(trow(a, d), trow(b, d)) for d in range(5)])
        chunks.append(ch)
    ch = []
    for a, b in pt[3:]:
        ch.append([(trow(a, d), trow(b, d)) for d in range(5)])
    chunks.append(ch)
    assert [len(c) for c in chunks] == PK
    assert all(sum(len(p) for p in c) == RK for c in chunks)
    return chunks


def _stationaries():
    vrow = lambda i, d: 3 * i + d
    trow = lambda t, d: 48 + 5 * t + d

    s1 = np.zeros((NRIN, 24), np.float16)
    for i in range(NV):
        for d in range(3):
            s1[vrow(i, d), i] = 1.0
    for t in range(NT):
        for d in range(5):
            s1[trow(t, d), 16 + t] = 1.0

    e4 = np.zeros((120, NRIN), np.float16)
    for j in range(4):
        e4[32 * j : 32 * j + 24, :] = s1.T

    p_ks, r_ks = [], []
    for ch in _chunk_pairs():
        p = np.zeros((NRIN, RK), np.float16)
        r = np.zeros((RK, 64), np.float16)
        row = 0
        for pl, comp in enumerate(ch):
            for ri, rj in comp:
                p[ri, row] = 1.0
                p[rj, row] = 1.0
                r[row, pl] = 1.0
                row += 1
        assert row == RK
        p_ks.append(p)
        r_ks.append(r)
    return s1, e4, p_ks, r_ks


def _build_nc():
    import concourse.bacc as bacc
    import concourse.bass as bass
    import concourse.tile as tile
    from concourse import mybir

    ACT = mybir.ActivationFunctionType
    ALU = mybir.AluOpType
    f32, f16 = mybir.dt.float32, mybir.dt.float16
    f32r = mybir.dt.float32r

    import concourse.hw_specs as hw_specs

    if not getattr(hw_specs, "_invx_patched", False):
        _orig_tables = hw_specs.get_activation_tables

        def _only_sqrt(module_arch):
            tabs = _orig_tables(module_arch)
            keep = "sqrt_and_others"
            assert keep in tabs
            # preserve set indices; empty the other sets so the
            # load-insertion pass can only pick the one covering
            # Sqrt+Square+Copy
            return {
                name: (funcs if name == keep else set())
                for name, funcs in tabs.items()
            }

        hw_specs.get_activation_tables = _only_sqrt
        import concourse.bacc as _bacc_mod

        _bacc_mod.get_activation_tables = _only_sqrt
        hw_specs._invx_patched = True

    nc = bacc.Bacc("TRN2", target_bir_lowering=False, debug=False, num_devices=N_CORES)

    eps_t = nc.alloc_sbuf_tensor("const-f32-eps2", [128, 1], f32)
    nc.gpsimd.memset(eps_t.ap(), EPS2)
    nc.const_aps.aps[(f32, EPS2)] = eps_t.ap()
    neg1_t = nc.alloc_sbuf_tensor("const-f32-neg1", [128, 1], f32)
    nc.gpsimd.memset(neg1_t.ap(), -1.0)
    nc.const_aps.aps[(f32, -1.0)] = neg1_t.ap()
    nc.all_engine_barrier()

    ht_ext = nc.declare_dram_parameter("hT", [NRIN, PADDED], f16, isOutput=False)
    s1_ext = nc.declare_dram_parameter("S1", [NRIN, 24], f16, isOutput=False)
    e4_ext = nc.declare_dram_parameter("E4", [120, NRIN], f16, isOutput=False)
    p_exts = [
        nc.declare_dram_parameter(f"P{k}", [NRIN, RK], f16, isOutput=False)
        for k in range(4)
    ]
    r_exts = [
        nc.declare_dram_parameter(f"R{k}", [RK, 64], f16, isOutput=False)
        for k in range(4)
    ]
    d_ext = nc.declare_dram_parameter("d", [164, PADDED], f16, isOutput=True)
    n_ext = nc.declare_dram_parameter("n", [24, PADDED], f32, isOutput=True)

    with tile.TileContext(nc) as tc:
        with (
            tc.tile_pool(name="const", bufs=1) as cpool,
            tc.tile_pool(name="x2", bufs=4) as xpool,
            tc.tile_pool(name="sq", bufs=8) as sqpool,
            tc.tile_pool(name="vu", bufs=4) as vupool,
            tc.tile_pool(name="squ", bufs=2) as squpool,
            tc.tile_pool(name="grp", bufs=2) as grppool,
            tc.tile_pool(name="sd", bufs=2) as sdpool,
            tc.tile_pool(name="ps_n2", bufs=1, space=bass.MemorySpace.PSUM) as ps_n2,
            tc.tile_pool(name="ps_re", bufs=1, space=bass.MemorySpace.PSUM) as ps_re,
            tc.tile_pool(name="ps_u", bufs=1, space=bass.MemorySpace.PSUM) as ps_u,
            tc.tile_pool(name="ps_d", bufs=1, space=bass.MemorySpace.PSUM) as ps_d,
        ):
            s1_t = cpool.tile([NRIN, 24], f16)
            nc.sync.dma_start(out=s1_t[:], in_=s1_ext[:])
            e4_t = cpool.tile([120, NRIN], f16)
            nc.sync.dma_start(out=e4_t[:], in_=e4_ext[:])
            p_ts, r_ts = [], []
            for k in range(4):
                p_t = cpool.tile([NRIN, RK], f16, tag=f"P{k}")
                nc.sync.dma_start(out=p_t[:], in_=p_exts[k][:])
                p_ts.append(p_t)
                r_t = cpool.tile([RK, 64], f16, tag=f"R{k}")
                nc.sync.dma_start(out=r_t[:], in_=r_exts[k][:])
                r_ts.append(r_t)

            xs = {}     # c -> (x2 tile, col offset)
            sqs = {}    # c -> sq tile
            vus = {}    # c -> vu tile
            squs = {}   # c -> (squ01, squ23)
            rinvs = {}  # g -> rinvg f16 tile

            def emit_x(c):
                """DMA x for chunks (c, c+1); c even."""
                if c >= N_CHUNKS:
                    return
                x2 = xpool.tile([NRIN, 2 * CHUNK], f16, tag="x2")
                nc.sync.dma_start(
                    out=x2[:], in_=ht_ext[:, c * CHUNK : (c + 2) * CHUNK]
                )
                xs[c] = (x2, 0)
                xs[c + 1] = (x2, CHUNK)

            def emit_sq(c):
                if c >= N_CHUNKS:
                    return
                x2, off = xs[c]
                xv = x2[:, off : off + CHUNK]
                sq_t = sqpool.tile([NRIN, CHUNK], f16, tag="sq")
                nc.gpsimd.tensor_mul(sq_t[:], xv, xv)
                sqs[c] = sq_t

            def emit_norm_chain(g):
                """n2 matmuls for group g + sqrt/recip/cast + norm DMA."""
                n2g = ps_n2.tile([128, CHUNK], f32, tag="n2g")
                for j in range(4):
                    c = 4 * g + j
                    nc.tensor.matmul(
                        n2g[32 * j : 32 * j + 24, :],
                        s1_t[:],
                        sqs[c][:],
                        tile_position=(0, 32 * j),
                    )
                    del sqs[c]
                normf = grppool.tile([128, CHUNK], f32, tag="normf")
                nc.scalar.activation(
                    normf[:], n2g[:], ACT.Sqrt, bias=EPS2, scale=1.0
                )
                rinvf = grppool.tile([128, CHUNK], f32, tag="rinvf")
                nc.vector.reciprocal_approx_fast(rinvf[:], normf[:])
                rinvg = grppool.tile([128, CHUNK], f16, tag="rinvg")
                nc.gpsimd.tensor_copy(rinvg[:], rinvf[:])
                rinvs[g] = rinvg
                src = normf[:].rearrange("(s r) c -> s r c", s=4)[:, 0:24, :]
                dst = n_ext[:, 4 * g * CHUNK : (4 * g + 4) * CHUNK].rearrange(
                    "r (s c) -> s r c", s=4
                )
                nc.sync.dma_start(out=dst, in_=src)

            def emit_rexp_vu(c):
                j = c % 4
                rinvg = rinvs[c // 4]
                rexp = ps_re.tile([NRIN, CHUNK], f32, tag="rexp")
                nc.tensor.matmul(
                    rexp[:],
                    e4_t[32 * j : 32 * j + 24, :],
                    rinvg[32 * j : 32 * j + 24, :],
                    tile_position=(32 * j, 0),
                )
                x2, off = xs[c]
                vu_t = vupool.tile([NRIN, CHUNK], f16, tag="vu")
                nc.vector.tensor_mul(vu_t[:], x2[:, off : off + CHUNK], rexp[:])
                vus[c] = vu_t

            def emit_u_phase(c):
                pair = []
                for half, (ka, kb) in enumerate([(0, 1), (2, 3)]):
                    u2 = ps_u.tile([RK, 2 * CHUNK], f32, tag=f"u{ka}{kb}")
                    nc.tensor.matmul(u2[:, 0:CHUNK], p_ts[ka][:], vus[c][:])
                    nc.tensor.matmul(
                        u2[:, CHUNK : 2 * CHUNK], p_ts[kb][:], vus[c][:]
                    )
                    squ2 = squpool.tile([RK, 2 * CHUNK], f16, tag=f"squ{ka}{kb}")
                    nc.scalar.activation(
                        squ2[:], u2[:], ACT.Square, bias=0.0, scale=1.0
                    )
                    pair.append(squ2)
                squs[c] = pair
                del vus[c]

            def emit_d(c):
                squ01, squ23 = squs.pop(c)
                dXY = ps_d.tile([128, 2 * CHUNK], f32, tag="dXY")
                for k in range(4):
                    half = k // 2
                    squ2 = squ01 if half == 0 else squ23
                    off = 64 * (k % 2)
                    nc.tensor.matmul(
                        dXY[off : off + 64, half * CHUNK : (half + 1) * CHUNK],
                        r_ts[k][:],
                        squ2[:, (k % 2) * CHUNK : (k % 2 + 1) * CHUNK],
                        tile_position=(0, off),
                    )
                s = sdpool.tile([128, 2 * CHUNK], f16, tag="s")
                nc.vector.tensor_scalar(
                    s[:], dXY[:], 0.5, -1.0, ALU.mult, ALU.add
                )
                cols = slice(c * CHUNK, (c + 1) * CHUNK)
                sX = s[:, 0:CHUNK].rearrange("(a b) c -> a b c", a=2)[:, 0:41, :]
                nc.sync.dma_start(
                    out=d_ext[0:82, cols].rearrange("(a b) c -> a b c", a=2),
                    in_=sX,
                )
                sY = s[:, CHUNK : 2 * CHUNK].rearrange("(a b) c -> a b c", a=2)[
                    :, 0:41, :
                ]
                nc.sync.dma_start(
                    out=d_ext[82:164, cols].rearrange("(a b) c -> a b c", a=2),
                    in_=sY,
                )

            # ---- prologue: group 0 ----
            emit_x(0)
            emit_x(2)
            for j in range(4):
                emit_sq(j)
            emit_norm_chain(0)

            dq = []
            for g in range(N_GROUPS):
                c0 = 4 * g
                emit_x(c0 + 4)
                emit_x(c0 + 6)
                for j in range(4):
                    emit_sq(c0 + 4 + j)
                emit_rexp_vu(c0 + 0)
                if dq:
                    emit_d(dq.pop(0))
                emit_rexp_vu(c0 + 1)
                emit_u_phase(c0 + 0)
                emit_rexp_vu(c0 + 2)
                if dq:
                    emit_d(dq.pop(0))
                emit_rexp_vu(c0 + 3)
                emit_u_phase(c0 + 1)
                emit_u_phase(c0 + 2)
                if g + 1 < N_GROUPS:
                    emit_norm_chain(g + 1)
                emit_d(c0 + 0)
                emit_u_phase(c0 + 3)
                emit_d(c0 + 1)
                dq += [c0 + 2, c0 + 3]

            while dq:
                emit_d(dq.pop(0))

    nc.compile()
    return nc


def _get_nc():
    if "nc" not in _CACHE:
        _CACHE["nc"] = _build_nc()
    return _CACHE["nc"]


def _make_in_map(shard, stat):
    """shard [n<=PADDED, 120] f32 -> hT [88, PADDED] f16 (vec+tens comps)."""
    n = shard.shape[0]
    buf = np.ones((NRIN, PADDED), np.float16)
    buf[:, :n] = shard[:, 32:120].astype(np.float16).T
    return {"hT": buf, **stat}


def _stat_map():
    s1, e4, p_ks, r_ks = _stationaries()
    stat = {"S1": s1, "E4": e4}
    for k in range(4):
        stat[f"P{k}"] = p_ks[k]
        stat[f"R{k}"] = r_ks[k]
    return stat


def _dev_row_maps():
    """Reference vdots/tdots order -> device d-row index."""
    vmap = np.empty(120, np.int64)
    for p in range(120):
        vmap[p] = (p // 40) * 41 + (p % 40)
    tmap = np.empty(28, np.int64)
    for q in range(28):
        tmap[q] = q * 41 + 40 if q < 3 else 123 + (q - 3)
    return vmap, tmap


def _assemble(out_block, res_c, shard):
    """Fill out_block [n, 204] from device outputs + host passthrough."""
    n = out_block.shape[0]
    vmap, tmap = _dev_row_maps()
    d = res_c["d"]  # [148, PADDED] f16
    nn = res_c["n"]  # [24, PADDED] f16
    out_block[:, 0:32] = shard[:, 0:32]  # scalars passthrough
    out_block[:, 32:48] = nn[0:16, :n].T.astype(np.float32)
    out_block[:, 48:56] = nn[16:24, :n].T.astype(np.float32)
    out_block[:, 56:176] = d[vmap, :n].T.astype(np.float32)
    out_block[:, 176:204] = d[tmap, :n].T.astype(np.float32)


def _run_pjrt(nc, in_maps):
    """Execute the Bass module on N_CORES devices via PJRT/shard_map with
    per-device buffer assembly and per-shard fetch (avoids giant host
    concats, which trip transfer limits on the axon path)."""
    import jax
    from jax.sharding import Mesh, NamedSharding, PartitionSpec
    from jax.experimental.shard_map import shard_map
    from concourse import mybir
    from concourse.bass2jax import (
        _bass_exec_p,
        install_neuronx_cc_hook,
        partition_id_tensor,
    )

    install_neuronx_cc_hook()
    partition_name = nc.partition_id_tensor.name if nc.partition_id_tensor else None
    in_names, out_names, out_avals = [], [], []
    for alloc in nc.m.functions[0].allocations:
        if not isinstance(alloc, mybir.MemoryLocationSet):
            continue
        name = alloc.memorylocations[0].name
        if alloc.kind == "ExternalInput":
            if name != partition_name:
                in_names.append(name)
        elif alloc.kind == "ExternalOutput":
            out_names.append(name)
            shape = tuple(alloc.tensor_shape)
            dtype = mybir.dt.np(alloc.dtype)
            out_avals.append(jax.core.ShapedArray(shape, dtype))
    n_params = len(in_names)
    n_outs = len(out_avals)
    all_in_names = list(in_names) + out_names
    if partition_name is not None:
        all_in_names.append(partition_name)
    donate = tuple(range(n_params, n_params + n_outs))

    def _body(*args):
        operands = list(args)
        if partition_name is not None:
            operands.append(partition_id_tensor())
        outs = _bass_exec_p.bind(
            *operands,
            out_avals=tuple(out_avals),
            in_names=tuple(all_in_names),
            out_names=tuple(out_names),
            lowering_input_output_aliases=(),
            sim_require_finite=True,
            sim_require_nnan=True,
            nc=nc,
        )
        return tuple(outs)

    devices = jax.devices()[:N_CORES]
    mesh = Mesh(np.asarray(devices), ("core",))
    sharding = NamedSharding(mesh, PartitionSpec("core"))
    fn = jax.jit(
        shard_map(
            _body,
            mesh=mesh,
            in_specs=(PartitionSpec("core"),) * (n_params + n_outs),
            out_specs=(PartitionSpec("core"),) * n_outs,
            check_rep=False,
        ),
        donate_argnums=donate,
        keep_unused=True,
    )

    def make_global(per_core_arrays):
        a0 = per_core_arrays[0]
        gshape = (N_CORES * a0.shape[0],) + a0.shape[1:]
        bufs = [
            jax.device_put(per_core_arrays[c], devices[c]) for c in range(N_CORES)
        ]
        return jax.make_array_from_single_device_arrays(gshape, sharding, bufs)

    g_ins = [
        make_global([np.asarray(in_maps[c][nm]) for c in range(N_CORES)])
        for nm in in_names
    ]
    g_zeros = [
        make_global([np.zeros(av.shape, av.dtype) for _ in range(N_CORES)])
        for av in out_avals
    ]
    outs = fn(*g_ins, *g_zeros)
    jax.block_until_ready(outs)

    results = [dict() for _ in range(N_CORES)]
    for i, nm in enumerate(out_names):
        shards = sorted(
            outs[i].addressable_shards, key=lambda s: devices.index(s.device)
        )
        for c, sh in enumerate(shards):
            results[c][nm] = np.asarray(sh.data)
    return results


def kernel(h):
    h = np.asarray(h, dtype=np.float32)
    assert h.shape == (N_ATOMS, 120)

    nc = _get_nc()
    stat = _stat_map()
    in_maps = [
        _make_in_map(h[c * PER_CORE : (c + 1) * PER_CORE], stat)
        for c in range(N_CORES)
    ]
    res = _run_pjrt(nc, in_maps)

    out = np.empty((N_ATOMS, NOUT), np.float32)
    for c in range(N_CORES):
        _assemble(
            out[c * PER_CORE : (c + 1) * PER_CORE],
            res[c],
            h[c * PER_CORE : (c + 1) * PER_CORE],
        )
    return out


# revision 17
# speedup vs baseline: 1.4087x; 1.4087x over previous
"""Trainium2 Bass kernel for EnhancedInvariantExtractor (v2).

Input  h [1_000_000, 120] f32:  per atom: 32 scalars | 16 vectors (l=1, dim 3)
                                | 8 tensors (l=2, dim 5).
Output [1_000_000, 204] f32: scalars(32) | vnorm(16) | tnorm(8) | vdots(120)
                             | tdots(28) (clipped pairwise cosines, upper
                             triangle, row-major).

Strategy (8 NeuronCores, data-parallel over atoms). v2 changes vs baseline:
- fp16 I/O: host ships hT [88, PADDED] f16 (vec+tens components only; the
  32 scalar columns are a pure identity passthrough done on host).
  Device outputs dots [148, PADDED] f16 + norms [24, PADDED] f16.
- Norm path is ONE ACT op: rinv = Rsqrt(n2 + eps) — the
  'reciprocal_sqrt_and_small' act table holds Rsqrt+Square+Copy, so no
  act-table churn. norm = n2 * rinv on DVE (grouped over 4 chunks).
- Pair-sum chunking repacked 5 -> 4 PE passes: [40 vec pairs + 1 tens pair]
  x3 + [25 tens pairs], 125 rows each. Dots: 4 passes into 2 PSUM banks at
  row offsets 0/64 (R stationaries padded to 64 cols with zeros so every
  PSUM row is written).
- PE work per chunk of 512 atoms: n2, rexp, u x4, d x4 = 10 passes of 512
  cols. Emission order software-pipelines across 4-chunk groups (d-passes
  of older chunks fill the rsqrt/vu latency windows) to keep the tensor
  engine gapless: TRN2 ramps 1.2 -> 2.4 GHz only after ~3us of continuous
  PE execution.
- Squares u^2 and the PSUM->SBUF dots moves (0.5*d - 1 affine folded in)
  are spread across ACT / DVE / Pool so no single elementwise engine
  exceeds the PE wall.
"""

import sys

sys.path.insert(0, "/opt/trn_rl_repo")

import numpy as np

N_ATOMS = 1_000_000
N_CORES = 8
PER_CORE = N_ATOMS // N_CORES  # 125_000
CHUNK = 512
N_CHUNKS = 248  # 62 groups of 4
PADDED = CHUNK * N_CHUNKS  # 126_976
N_GROUPS = N_CHUNKS // 4
NV, NT = 16, 8
NRIN = 88  # device input rows: vec comps 48 | tens comps 40
NPAIR = 148
NOUT = 204
EPS2 = 1e-12
# pair chunking: 3 chunks of [40 v-pairs + 1 t-pair] + 1 chunk of [25 t-pairs]
PK = [41, 41, 41, 25]
RK = 125

_CACHE = {}


def _pairs():
    pv = [(i, j) for i in range(NV) for j in range(i + 1, NV)]
    pt = [(a, b) for a in range(NT) for b in range(a + 1, NT)]
    return pv, pt


def _chunk_pairs():
    """Four u-chunks; each entry is a list of (rows_i, rows_j) component-row
    tuples. Chunks 0-2: v-pairs 40k..40k+40 then t-pair k. Chunk 3:
    t-pairs 3..28."""
    pv, pt = _pairs()
    vrow = lambda i, d: 3 * i + d
    trow = lambda t, d: 48 + 5 * t + d
    chunks = []
    for k in range(3):
        ch = []
        for i, j in pv[40 * k : 40 * k + 40]:
            ch.append([(vrow(i, d), vrow(j, d)) for d in range(3)])
        a, b = pt[k]
        ch.append([(trow(a, d), trow(b, d)) for d in range(5)])
        chunks.append(ch)
    ch = []
    for a, b in pt[3:]:
        ch.append([(trow(a, d), trow(b, d)) for d in range(5)])
    chunks.append(ch)
    assert [len(c) for c in chunks] == PK
    assert all(sum(len(p) for p in c) == RK for c in chunks)
    return chunks


def _stationaries():
    vrow = lambda i, d: 3 * i + d
    trow = lambda t, d: 48 + 5 * t + d

    s1 = np.zeros((NRIN, 24), np.float16)
    for i in range(NV):
        for d in range(3):
            s1[vrow(i, d), i] = 1.0
    for t in range(NT):
        for d in range(5):
            s1[trow(t, d), 16 + t] = 1.0

    e4 = np.zeros((120, NRIN), np.float16)
    for j in range(4):
        e4[32 * j : 32 * j + 24, :] = s1.T

    p_ks, r_ks = [], []
    for ch in _chunk_pairs():
        p = np.zeros((NRIN, RK), np.float16)
        r = np.zeros((RK, 64), np.float16)
        row = 0
        for pl, comp in enumerate(ch):
            for ri, rj in comp:
                p[ri, row] = 1.0
                p[rj, row] = 1.0
                r[row, pl] = 1.0
                row += 1
        assert row == RK
        p_ks.append(p)
        r_ks.append(r)
    return s1, e4, p_ks, r_ks


def _build_nc():
    import concourse.bacc as bacc
    import concourse.bass as bass
    import concourse.tile as tile
    from concourse import mybir

    ACT = mybir.ActivationFunctionType
    ALU = mybir.AluOpType
    f32, f16 = mybir.dt.float32, mybir.dt.float16
    f32r = mybir.dt.float32r

    import concourse.hw_specs as hw_specs

    if not getattr(hw_specs, "_invx_patched", False):
        _orig_tables = hw_specs.get_activation_tables

        def _only_sqrt(module_arch):
            tabs = _orig_tables(module_arch)
            keep = "sqrt_and_others"
            assert keep in tabs
            # preserve set indices; empty the other sets so the
            # load-insertion pass can only pick the one covering
            # Sqrt+Square+Copy
            return {
                name: (funcs if name == keep else set())
                for name, funcs in tabs.items()
            }

        hw_specs.get_activation_tables = _only_sqrt
        import concourse.bacc as _bacc_mod

        _bacc_mod.get_activation_tables = _only_sqrt
        hw_specs._invx_patched = True

    nc = bacc.Bacc("TRN2", target_bir_lowering=False, debug=False, num_devices=N_CORES)

    eps_t = nc.alloc_sbuf_tensor("const-f32-eps2", [128, 1], f32)
    nc.gpsimd.memset(eps_t.ap(), EPS2)
    nc.const_aps.aps[(f32, EPS2)] = eps_t.ap()
    neg1_t = nc.alloc_sbuf_tensor("const-f32-neg1", [128, 1], f32)
    nc.gpsimd.memset(neg1_t.ap(), -1.0)
    nc.const_aps.aps[(f32, -1.0)] = neg1_t.ap()
    nc.all_engine_barrier()

    ht_ext = nc.declare_dram_parameter("hT", [NRIN, PADDED], f16, isOutput=False)
    s1_ext = nc.declare_dram_parameter("S1", [NRIN, 24], f16, isOutput=False)
    e4_ext = nc.declare_dram_parameter("E4", [120, NRIN], f16, isOutput=False)
    p_exts = [
        nc.declare_dram_parameter(f"P{k}", [NRIN, RK], f16, isOutput=False)
        for k in range(4)
    ]
    r_exts = [
        nc.declare_dram_parameter(f"R{k}", [RK, 64], f16, isOutput=False)
        for k in range(4)
    ]
    dx_ext = nc.declare_dram_parameter("dx", [128, PADDED], f16, isOutput=True)
    dy_ext = nc.declare_dram_parameter("dy", [128, PADDED], f16, isOutput=True)
    n_ext = nc.declare_dram_parameter("n", [128, N_GROUPS * CHUNK], f32, isOutput=True)

    with tile.TileContext(nc) as tc:
        with (
            tc.tile_pool(name="const", bufs=1) as cpool,
            tc.tile_pool(name="x2", bufs=4) as xpool,
            tc.tile_pool(name="sq", bufs=8) as sqpool,
            tc.tile_pool(name="vu", bufs=4) as vupool,
            tc.tile_pool(name="squ", bufs=2) as squpool,
            tc.tile_pool(name="grp", bufs=2) as grppool,
            tc.tile_pool(name="sd", bufs=2) as sdpool,
            tc.tile_pool(name="ps_n2", bufs=1, space=bass.MemorySpace.PSUM) as ps_n2,
            tc.tile_pool(name="ps_re", bufs=1, space=bass.MemorySpace.PSUM) as ps_re,
            tc.tile_pool(name="ps_u", bufs=1, space=bass.MemorySpace.PSUM) as ps_u,
            tc.tile_pool(name="ps_d", bufs=1, space=bass.MemorySpace.PSUM) as ps_d,
        ):
            s1_t = cpool.tile([NRIN, 24], f16)
            nc.sync.dma_start(out=s1_t[:], in_=s1_ext[:])
            e4_t = cpool.tile([120, NRIN], f16)
            nc.sync.dma_start(out=e4_t[:], in_=e4_ext[:])
            p_ts, r_ts = [], []
            for k in range(4):
                p_t = cpool.tile([NRIN, RK], f16, tag=f"P{k}")
                nc.sync.dma_start(out=p_t[:], in_=p_exts[k][:])
                p_ts.append(p_t)
                r_t = cpool.tile([RK, 64], f16, tag=f"R{k}")
                nc.sync.dma_start(out=r_t[:], in_=r_exts[k][:])
                r_ts.append(r_t)

            xs = {}     # c -> (x2 tile, col offset)
            sqs = {}    # c -> sq tile
            vus = {}    # c -> vu tile
            squs = {}   # c -> (squ01, squ23)
            rinvs = {}  # g -> rinvg f16 tile

            def emit_x(c):
                """DMA x for chunks (c, c+1); c even."""
                if c >= N_CHUNKS:
                    return
                x2 = xpool.tile([NRIN, 2 * CHUNK], f16, tag="x2")
                nc.sync.dma_start(
                    out=x2[:], in_=ht_ext[:, c * CHUNK : (c + 2) * CHUNK]
                )
                xs[c] = (x2, 0)
                xs[c + 1] = (x2, CHUNK)

            def emit_sq(c):
                if c >= N_CHUNKS:
                    return
                x2, off = xs[c]
                xv = x2[:, off : off + CHUNK]
                sq_t = sqpool.tile([NRIN, CHUNK], f16, tag="sq")
                nc.gpsimd.tensor_mul(sq_t[:], xv, xv)
                sqs[c] = sq_t

            def emit_norm_chain(g):
                """n2 matmuls for group g + sqrt/recip/cast + norm DMA."""
                n2g = ps_n2.tile([128, CHUNK], f32, tag="n2g")
                for j in range(4):
                    c = 4 * g + j
                    nc.tensor.matmul(
                        n2g[32 * j : 32 * j + 24, :],
                        s1_t[:],
                        sqs[c][:],
                        tile_position=(0, 32 * j),
                    )
                    del sqs[c]
                normf = grppool.tile([128, CHUNK], f32, tag="normf")
                nc.scalar.activation(
                    normf[:], n2g[:], ACT.Sqrt, bias=EPS2, scale=1.0
                )
                rinvf = grppool.tile([128, CHUNK], f32, tag="rinvf")
                nc.vector.reciprocal_approx_fast(rinvf[:], normf[:])
                rinvg = grppool.tile([128, CHUNK], f16, tag="rinvg")
                nc.gpsimd.tensor_copy(rinvg[:], rinvf[:])
                rinvs[g] = rinvg
                # full-tile 2-dim store (partial/3-dim DMAs pin to one queue)
                nc.sync.dma_start(
                    out=n_ext[:, g * CHUNK : (g + 1) * CHUNK], in_=normf[:]
                )

            def emit_rexp_vu(c):
                j = c % 4
                rinvg = rinvs[c // 4]
                rexp = ps_re.tile([NRIN, CHUNK], f32, tag="rexp")
                nc.tensor.matmul(
                    rexp[:],
                    e4_t[32 * j : 32 * j + 24, :],
                    rinvg[32 * j : 32 * j + 24, :],
                    tile_position=(32 * j, 0),
                )
                x2, off = xs[c]
                vu_t = vupool.tile([NRIN, CHUNK], f16, tag="vu")
                nc.vector.tensor_mul(vu_t[:], x2[:, off : off + CHUNK], rexp[:])
                vus[c] = vu_t

            def emit_u_phase(c):
                pair = []
                for half, (ka, kb) in enumerate([(0, 1), (2, 3)]):
                    u2 = ps_u.tile([RK, 2 * CHUNK], f32, tag=f"u{ka}{kb}")
                    nc.tensor.matmul(u2[:, 0:CHUNK], p_ts[ka][:], vus[c][:])
                    nc.tensor.matmul(
                        u2[:, CHUNK : 2 * CHUNK], p_ts[kb][:], vus[c][:]
                    )
                    squ2 = squpool.tile([RK, 2 * CHUNK], f16, tag=f"squ{ka}{kb}")
                    nc.scalar.activation(
                        squ2[:], u2[:], ACT.Square, bias=0.0, scale=1.0
                    )
                    pair.append(squ2)
                squs[c] = pair
                del vus[c]

            sstate = {}

            def emit_d(c):
                squ01, squ23 = squs.pop(c)
                dXY = ps_d.tile([128, 2 * CHUNK], f32, tag="dXY")
                for k in range(4):
                    half = k // 2
                    squ2 = squ01 if half == 0 else squ23
                    off = 64 * (k % 2)
                    nc.tensor.matmul(
                        dXY[off : off + 64, half * CHUNK : (half + 1) * CHUNK],
                        r_ts[k][:],
                        squ2[:, (k % 2) * CHUNK : (k % 2 + 1) * CHUNK],
                        tile_position=(0, off),
                    )
                # s layout [128, 4*CHUNK]: cols [X(c0) | X(c1) | Y(c0) | Y(c1)]
                p = c // 2
                if c % 2 == 0:
                    s_new = sdpool.tile([128, 4 * CHUNK], f16, tag="s")
                    sstate[p] = s_new
                s = sstate[p]
                w = (c % 2) * CHUNK
                sview = s[:].rearrange("p (a c) -> p a c", a=2)[:, :, w : w + CHUNK]
                dview = dXY[:].rearrange("p (a c) -> p a c", a=2)
                nc.vector.tensor_scalar(
                    sview, dview, 0.5, -1.0, ALU.mult, ALU.add
                )
                if c % 2 == 1:
                    # baseline-shaped 2-dim stores: [128, 1024] full rows
                    # (junk rows shipped; host ignores them)
                    cols = slice((c - 1) * CHUNK, (c + 1) * CHUNK)
                    nc.sync.dma_start(
                        out=dx_ext[:, cols], in_=s[:, 0 : 2 * CHUNK]
                    )
                    nc.sync.dma_start(
                        out=dy_ext[:, cols], in_=s[:, 2 * CHUNK : 4 * CHUNK]
                    )
                    del sstate[p]

            # ---- prologue: group 0 ----
            emit_x(0)
            emit_x(2)
            for j in range(4):
                emit_sq(j)
            emit_norm_chain(0)

            dq = []
            for g in range(N_GROUPS):
                c0 = 4 * g
                emit_x(c0 + 4)
                emit_x(c0 + 6)
                for j in range(4):
                    emit_sq(c0 + 4 + j)
                emit_rexp_vu(c0 + 0)
                if dq:
                    emit_d(dq.pop(0))
                emit_rexp_vu(c0 + 1)
                emit_u_phase(c0 + 0)
                emit_rexp_vu(c0 + 2)
                if dq:
                    emit_d(dq.pop(0))
                emit_rexp_vu(c0 + 3)
                emit_u_phase(c0 + 1)
                emit_u_phase(c0 + 2)
                if g + 1 < N_GROUPS:
                    emit_norm_chain(g + 1)
                emit_d(c0 + 0)
                emit_u_phase(c0 + 3)
                emit_d(c0 + 1)
                dq += [c0 + 2, c0 + 3]

            while dq:
                emit_d(dq.pop(0))

    nc.compile()
    return nc


def _get_nc():
    if "nc" not in _CACHE:
        _CACHE["nc"] = _build_nc()
    return _CACHE["nc"]


def _make_in_map(shard, stat):
    """shard [n<=PADDED, 120] f32 -> hT [88, PADDED] f16 (vec+tens comps)."""
    n = shard.shape[0]
    buf = np.ones((NRIN, PADDED), np.float16)
    buf[:, :n] = shard[:, 32:120].astype(np.float16).T
    return {"hT": buf, **stat}


def _stat_map():
    s1, e4, p_ks, r_ks = _stationaries()
    stat = {"S1": s1, "E4": e4}
    for k in range(4):
        stat[f"P{k}"] = p_ks[k]
        stat[f"R{k}"] = r_ks[k]
    return stat


def _dev_row_maps():
    """Reference vdots/tdots order -> device d-row index."""
    vmap = np.empty(120, np.int64)
    for p in range(120):
        vmap[p] = (p // 40) * 41 + (p % 40)
    tmap = np.empty(28, np.int64)
    for q in range(28):
        tmap[q] = q * 41 + 40 if q < 3 else 123 + (q - 3)
    return vmap, tmap


def _assemble(out_block, res_c, shard):
    """Fill out_block [n, 204] from device outputs + host passthrough."""
    n = out_block.shape[0]
    vmap, tmap = _dev_row_maps()
    dx, dy = res_c["dx"], res_c["dy"]  # [128, PADDED] f16 each
    d = np.concatenate(
        [dx[0:41], dx[64:105], dy[0:41], dy[64:89]], axis=0
    )  # [148, PADDED] device pair order
    # n: [128, N_GROUPS*512] f32; chunk 4g+j strip at rows 32j:32j+24,
    # cols g*512:(g+1)*512 -> un-strip to [24, PADDED]
    ng = res_c["n"].reshape(4, 32, N_GROUPS, CHUNK)
    nn = ng.transpose(1, 2, 0, 3).reshape(32, PADDED)
    out_block[:, 0:32] = shard[:, 0:32]  # scalars passthrough
    out_block[:, 32:48] = nn[0:16, :n].T.astype(np.float32)
    out_block[:, 48:56] = nn[16:24, :n].T.astype(np.float32)
    out_block[:, 56:176] = d[vmap, :n].T.astype(np.float32)
    out_block[:, 176:204] = d[tmap, :n].T.astype(np.float32)


def _run_pjrt(nc, in_maps):
    """Execute the Bass module on N_CORES devices via PJRT/shard_map with
    per-device buffer assembly and per-shard fetch (avoids giant host
    concats, which trip transfer limits on the axon path)."""
    import jax
    from jax.sharding import Mesh, NamedSharding, PartitionSpec
    from jax.experimental.shard_map import shard_map
    from concourse import mybir
    from concourse.bass2jax import (
        _bass_exec_p,
        install_neuronx_cc_hook,
        partition_id_tensor,
    )

    install_neuronx_cc_hook()
    partition_name = nc.partition_id_tensor.name if nc.partition_id_tensor else None
    in_names, out_names, out_avals = [], [], []
    for alloc in nc.m.functions[0].allocations:
        if not isinstance(alloc, mybir.MemoryLocationSet):
            continue
        name = alloc.memorylocations[0].name
        if alloc.kind == "ExternalInput":
            if name != partition_name:
                in_names.append(name)
        elif alloc.kind == "ExternalOutput":
            out_names.append(name)
            shape = tuple(alloc.tensor_shape)
            dtype = mybir.dt.np(alloc.dtype)
            out_avals.append(jax.core.ShapedArray(shape, dtype))
    n_params = len(in_names)
    n_outs = len(out_avals)
    all_in_names = list(in_names) + out_names
    if partition_name is not None:
        all_in_names.append(partition_name)
    donate = tuple(range(n_params, n_params + n_outs))

    def _body(*args):
        operands = list(args)
        if partition_name is not None:
            operands.append(partition_id_tensor())
        outs = _bass_exec_p.bind(
            *operands,
            out_avals=tuple(out_avals),
            in_names=tuple(all_in_names),
            out_names=tuple(out_names),
            lowering_input_output_aliases=(),
            sim_require_finite=True,
            sim_require_nnan=True,
            nc=nc,
        )
        return tuple(outs)

    devices = jax.devices()[:N_CORES]
    mesh = Mesh(np.asarray(devices), ("core",))
    sharding = NamedSharding(mesh, PartitionSpec("core"))
    fn = jax.jit(
        shard_map(
            _body,
            mesh=mesh,
            in_specs=(PartitionSpec("core"),) * (n_params + n_outs),
            out_specs=(PartitionSpec("core"),) * n_outs,
            check_rep=False,
        ),
        donate_argnums=donate,
        keep_unused=True,
    )

    def make_global(per_core_arrays):
        a0 = per_core_arrays[0]
        gshape = (N_CORES * a0.shape[0],) + a0.shape[1:]
        bufs = [
            jax.device_put(per_core_arrays[c], devices[c]) for c in range(N_CORES)
        ]
        return jax.make_array_from_single_device_arrays(gshape, sharding, bufs)

    g_ins = [
        make_global([np.asarray(in_maps[c][nm]) for c in range(N_CORES)])
        for nm in in_names
    ]
    g_zeros = [
        make_global([np.zeros(av.shape, av.dtype) for _ in range(N_CORES)])
        for av in out_avals
    ]
    outs = fn(*g_ins, *g_zeros)
    jax.block_until_ready(outs)

    results = [dict() for _ in range(N_CORES)]
    for i, nm in enumerate(out_names):
        shards = sorted(
            outs[i].addressable_shards, key=lambda s: devices.index(s.device)
        )
        for c, sh in enumerate(shards):
            results[c][nm] = np.asarray(sh.data)
    return results


def kernel(h):
    h = np.asarray(h, dtype=np.float32)
    assert h.shape == (N_ATOMS, 120)

    nc = _get_nc()
    stat = _stat_map()
    in_maps = [
        _make_in_map(h[c * PER_CORE : (c + 1) * PER_CORE], stat)
        for c in range(N_CORES)
    ]
    res = _run_pjrt(nc, in_maps)

    out = np.empty((N_ATOMS, NOUT), np.float32)
    for c in range(N_CORES):
        _assemble(
            out[c * PER_CORE : (c + 1) * PER_CORE],
            res[c],
            h[c * PER_CORE : (c + 1) * PER_CORE],
        )
    return out


# revision 18
# speedup vs baseline: 1.5769x; 1.1194x over previous
"""Trainium2 Bass kernel for EnhancedInvariantExtractor (v2).

Input  h [1_000_000, 120] f32:  per atom: 32 scalars | 16 vectors (l=1, dim 3)
                                | 8 tensors (l=2, dim 5).
Output [1_000_000, 204] f32: scalars(32) | vnorm(16) | tnorm(8) | vdots(120)
                             | tdots(28) (clipped pairwise cosines, upper
                             triangle, row-major).

Strategy (8 NeuronCores, data-parallel over atoms). v2 changes vs baseline:
- fp16 I/O: host ships hT [88, PADDED] f16 (vec+tens components only; the
  32 scalar columns are a pure identity passthrough done on host).
  Device outputs dots [148, PADDED] f16 + norms [24, PADDED] f16.
- Norm path is ONE ACT op: rinv = Rsqrt(n2 + eps) — the
  'reciprocal_sqrt_and_small' act table holds Rsqrt+Square+Copy, so no
  act-table churn. norm = n2 * rinv on DVE (grouped over 4 chunks).
- Pair-sum chunking repacked 5 -> 4 PE passes: [40 vec pairs + 1 tens pair]
  x3 + [25 tens pairs], 125 rows each. Dots: 4 passes into 2 PSUM banks at
  row offsets 0/64 (R stationaries padded to 64 cols with zeros so every
  PSUM row is written).
- PE work per chunk of 512 atoms: n2, rexp, u x4, d x4 = 10 passes of 512
  cols. Emission order software-pipelines across 4-chunk groups (d-passes
  of older chunks fill the rsqrt/vu latency windows) to keep the tensor
  engine gapless: TRN2 ramps 1.2 -> 2.4 GHz only after ~3us of continuous
  PE execution.
- Squares u^2 and the PSUM->SBUF dots moves (0.5*d - 1 affine folded in)
  are spread across ACT / DVE / Pool so no single elementwise engine
  exceeds the PE wall.
"""

import sys

sys.path.insert(0, "/opt/trn_rl_repo")

import numpy as np

N_ATOMS = 1_000_000
N_CORES = 8
PER_CORE = N_ATOMS // N_CORES  # 125_000
CHUNK = 512
N_CHUNKS = 248  # 62 groups of 4
PADDED = CHUNK * N_CHUNKS  # 126_976
N_GROUPS = N_CHUNKS // 4
NV, NT = 16, 8
NRIN = 88  # device input rows: vec comps 48 | tens comps 40
NPAIR = 148
NOUT = 204
EPS2 = 1e-12
# pair chunking: 3 chunks of [40 v-pairs + 1 t-pair] + 1 chunk of [25 t-pairs]
PK = [41, 41, 41, 25]
RK = 125

_CACHE = {}


def _pairs():
    pv = [(i, j) for i in range(NV) for j in range(i + 1, NV)]
    pt = [(a, b) for a in range(NT) for b in range(a + 1, NT)]
    return pv, pt


def _chunk_pairs():
    """Four u-chunks; each entry is a list of (rows_i, rows_j) component-row
    tuples. Chunks 0-2: v-pairs 40k..40k+40 then t-pair k. Chunk 3:
    t-pairs 3..28."""
    pv, pt = _pairs()
    vrow = lambda i, d: 3 * i + d
    trow = lambda t, d: 48 + 5 * t + d
    chunks = []
    for k in range(3):
        ch = []
        for i, j in pv[40 * k : 40 * k + 40]:
            ch.append([(vrow(i, d), vrow(j, d)) for d in range(3)])
        a, b = pt[k]
        ch.append([(trow(a, d), trow(b, d)) for d in range(5)])
        chunks.append(ch)
    ch = []
    for a, b in pt[3:]:
        ch.append([(trow(a, d), trow(b, d)) for d in range(5)])
    chunks.append(ch)
    assert [len(c) for c in chunks] == PK
    assert all(sum(len(p) for p in c) == RK for c in chunks)
    return chunks


def _stationaries():
    vrow = lambda i, d: 3 * i + d
    trow = lambda t, d: 48 + 5 * t + d

    s1 = np.zeros((NRIN, 24), np.float16)
    for i in range(NV):
        for d in range(3):
            s1[vrow(i, d), i] = 1.0
    for t in range(NT):
        for d in range(5):
            s1[trow(t, d), 16 + t] = 1.0

    e4 = np.zeros((120, NRIN), np.float16)
    for j in range(4):
        e4[32 * j : 32 * j + 24, :] = s1.T

    p_ks, r_ks = [], []
    for ch in _chunk_pairs():
        p = np.zeros((NRIN, RK), np.float16)
        r = np.zeros((RK, 64), np.float16)
        row = 0
        for pl, comp in enumerate(ch):
            for ri, rj in comp:
                p[ri, row] = 1.0
                p[rj, row] = 1.0
                r[row, pl] = 1.0
                row += 1
        assert row == RK
        p_ks.append(p)
        r_ks.append(r)
    return s1, e4, p_ks, r_ks


def _build_nc():
    import concourse.bacc as bacc
    import concourse.bass as bass
    import concourse.tile as tile
    from concourse import mybir

    ACT = mybir.ActivationFunctionType
    ALU = mybir.AluOpType
    f32, f16 = mybir.dt.float32, mybir.dt.float16
    f32r = mybir.dt.float32r

    import concourse.hw_specs as hw_specs

    if not getattr(hw_specs, "_invx_patched", False):
        _orig_tables = hw_specs.get_activation_tables

        def _only_sqrt(module_arch):
            tabs = _orig_tables(module_arch)
            keep = "sqrt_and_others"
            assert keep in tabs
            # preserve set indices; empty the other sets so the
            # load-insertion pass can only pick the one covering
            # Sqrt+Square+Copy
            return {
                name: (funcs if name == keep else set())
                for name, funcs in tabs.items()
            }

        hw_specs.get_activation_tables = _only_sqrt
        import concourse.bacc as _bacc_mod

        _bacc_mod.get_activation_tables = _only_sqrt
        hw_specs._invx_patched = True

    nc = bacc.Bacc("TRN2", target_bir_lowering=False, debug=False, num_devices=N_CORES)

    eps_t = nc.alloc_sbuf_tensor("const-f32-eps2", [128, 1], f32)
    nc.gpsimd.memset(eps_t.ap(), EPS2)
    nc.const_aps.aps[(f32, EPS2)] = eps_t.ap()
    neg1_t = nc.alloc_sbuf_tensor("const-f32-neg1", [128, 1], f32)
    nc.gpsimd.memset(neg1_t.ap(), -1.0)
    nc.const_aps.aps[(f32, -1.0)] = neg1_t.ap()
    nc.all_engine_barrier()

    ht_ext = nc.declare_dram_parameter("hT", [NRIN, PADDED], f16, isOutput=False)
    s1_ext = nc.declare_dram_parameter("S1", [NRIN, 24], f16, isOutput=False)
    e4_ext = nc.declare_dram_parameter("E4", [120, NRIN], f16, isOutput=False)
    p_exts = [
        nc.declare_dram_parameter(f"P{k}", [NRIN, RK], f16, isOutput=False)
        for k in range(4)
    ]
    r_exts = [
        nc.declare_dram_parameter(f"R{k}", [RK, 64], f16, isOutput=False)
        for k in range(4)
    ]
    dx_ext = nc.declare_dram_parameter("dx", [128, PADDED], f16, isOutput=True)
    dy_ext = nc.declare_dram_parameter("dy", [128, PADDED], f16, isOutput=True)
    n_ext = nc.declare_dram_parameter("n", [128, N_GROUPS * CHUNK], f32, isOutput=True)

    with tile.TileContext(nc) as tc:
        with (
            tc.tile_pool(name="const", bufs=1) as cpool,
            tc.tile_pool(name="x2", bufs=4) as xpool,
            tc.tile_pool(name="sq", bufs=8) as sqpool,
            tc.tile_pool(name="vu", bufs=4) as vupool,
            tc.tile_pool(name="squ", bufs=4) as squpool,
            tc.tile_pool(name="grp", bufs=2) as grppool,
            tc.tile_pool(name="sd", bufs=2) as sdpool,
            tc.tile_pool(name="ps_n2", bufs=1, space=bass.MemorySpace.PSUM) as ps_n2,
            tc.tile_pool(name="ps_re", bufs=1, space=bass.MemorySpace.PSUM) as ps_re,
            tc.tile_pool(name="ps_u", bufs=1, space=bass.MemorySpace.PSUM) as ps_u,
            tc.tile_pool(name="ps_d", bufs=1, space=bass.MemorySpace.PSUM) as ps_d,
        ):
            s1_t = cpool.tile([NRIN, 24], f16)
            nc.sync.dma_start(out=s1_t[:], in_=s1_ext[:])
            e4_t = cpool.tile([120, NRIN], f16)
            nc.sync.dma_start(out=e4_t[:], in_=e4_ext[:])
            p_ts, r_ts = [], []
            for k in range(4):
                p_t = cpool.tile([NRIN, RK], f16, tag=f"P{k}")
                nc.sync.dma_start(out=p_t[:], in_=p_exts[k][:])
                p_ts.append(p_t)
                r_t = cpool.tile([RK, 64], f16, tag=f"R{k}")
                nc.sync.dma_start(out=r_t[:], in_=r_exts[k][:])
                r_ts.append(r_t)

            xs = {}     # c -> (x2 tile, col offset)
            sqs = {}    # c -> sq tile
            vus = {}    # c -> vu tile
            squs = {}   # c -> (squ01, squ23)
            rinvs = {}  # g -> rinvg f16 tile

            def emit_x(c):
                """DMA x for chunks (c, c+1); c even."""
                if c >= N_CHUNKS:
                    return
                x2 = xpool.tile([NRIN, 2 * CHUNK], f16, tag="x2")
                nc.sync.dma_start(
                    out=x2[:], in_=ht_ext[:, c * CHUNK : (c + 2) * CHUNK]
                )
                xs[c] = (x2, 0)
                xs[c + 1] = (x2, CHUNK)

            def emit_sq(c):
                if c >= N_CHUNKS:
                    return
                x2, off = xs[c]
                xv = x2[:, off : off + CHUNK]
                sq_t = sqpool.tile([NRIN, CHUNK], f16, tag="sq")
                nc.gpsimd.tensor_mul(sq_t[:], xv, xv)
                sqs[c] = sq_t

            def emit_norm_chain(g):
                """n2 matmuls for group g + sqrt/recip/cast + norm DMA."""
                n2g = ps_n2.tile([128, CHUNK], f32, tag="n2g")
                for j in range(4):
                    c = 4 * g + j
                    nc.tensor.matmul(
                        n2g[32 * j : 32 * j + 24, :],
                        s1_t[:],
                        sqs[c][:],
                        tile_position=(0, 32 * j),
                    )
                    del sqs[c]
                normf = grppool.tile([128, CHUNK], f32, tag="normf")
                nc.scalar.activation(
                    normf[:], n2g[:], ACT.Sqrt, bias=EPS2, scale=1.0
                )
                rinvf = grppool.tile([128, CHUNK], f32, tag="rinvf")
                nc.vector.reciprocal_approx_fast(rinvf[:], normf[:])
                rinvg = grppool.tile([128, CHUNK], f16, tag="rinvg")
                nc.gpsimd.tensor_copy(rinvg[:], rinvf[:])
                rinvs[g] = rinvg
                # full-tile 2-dim store (partial/3-dim DMAs pin to one queue)
                nc.sync.dma_start(
                    out=n_ext[:, g * CHUNK : (g + 1) * CHUNK], in_=normf[:]
                )

            def emit_rexp_vu(c):
                j = c % 4
                rinvg = rinvs[c // 4]
                rexp = ps_re.tile([NRIN, CHUNK], f32, tag="rexp")
                nc.tensor.matmul(
                    rexp[:],
                    e4_t[32 * j : 32 * j + 24, :],
                    rinvg[32 * j : 32 * j + 24, :],
                    tile_position=(32 * j, 0),
                )
                x2, off = xs[c]
                vu_t = vupool.tile([NRIN, CHUNK], f16, tag="vu")
                nc.vector.tensor_mul(vu_t[:], x2[:, off : off + CHUNK], rexp[:])
                vus[c] = vu_t

            def emit_u_phase(c):
                pair = []
                for half, (ka, kb) in enumerate([(0, 1), (2, 3)]):
                    u2 = ps_u.tile([RK, 2 * CHUNK], f32, tag=f"u{ka}{kb}")
                    nc.tensor.matmul(u2[:, 0:CHUNK], p_ts[ka][:], vus[c][:])
                    nc.tensor.matmul(
                        u2[:, CHUNK : 2 * CHUNK], p_ts[kb][:], vus[c][:]
                    )
                    squ2 = squpool.tile([RK, 2 * CHUNK], f16, tag=f"squ{ka}{kb}")
                    nc.scalar.activation(
                        squ2[:], u2[:], ACT.Square, bias=0.0, scale=1.0
                    )
                    pair.append(squ2)
                squs[c] = pair
                del vus[c]

            sstate = {}

            def emit_d(c):
                squ01, squ23 = squs.pop(c)
                dXY = ps_d.tile([128, 2 * CHUNK], f32, tag="dXY")
                for k in range(4):
                    half = k // 2
                    squ2 = squ01 if half == 0 else squ23
                    off = 64 * (k % 2)
                    nc.tensor.matmul(
                        dXY[off : off + 64, half * CHUNK : (half + 1) * CHUNK],
                        r_ts[k][:],
                        squ2[:, (k % 2) * CHUNK : (k % 2 + 1) * CHUNK],
                        tile_position=(0, off),
                    )
                # s layout [128, 4*CHUNK]: cols [X(c0) | X(c1) | Y(c0) | Y(c1)]
                p = c // 2
                if c % 2 == 0:
                    s_new = sdpool.tile([128, 4 * CHUNK], f16, tag="s")
                    sstate[p] = s_new
                s = sstate[p]
                w = (c % 2) * CHUNK
                sview = s[:].rearrange("p (a c) -> p a c", a=2)[:, :, w : w + CHUNK]
                dview = dXY[:].rearrange("p (a c) -> p a c", a=2)
                nc.vector.tensor_scalar(
                    sview, dview, 0.5, -1.0, ALU.mult, ALU.add
                )
                if c % 2 == 1:
                    # baseline-shaped 2-dim stores: [128, 1024] full rows
                    # (junk rows shipped; host ignores them)
                    cols = slice((c - 1) * CHUNK, (c + 1) * CHUNK)
                    nc.sync.dma_start(
                        out=dx_ext[:, cols], in_=s[:, 0 : 2 * CHUNK]
                    )
                    nc.sync.dma_start(
                        out=dy_ext[:, cols], in_=s[:, 2 * CHUNK : 4 * CHUNK]
                    )
                    del sstate[p]

            # ---- prologue: group 0 ----
            emit_x(0)
            emit_x(2)
            for j in range(4):
                emit_sq(j)
            emit_norm_chain(0)
            emit_rexp_vu(0)

            # steady-state PE stream per group g (slot schedule keeps every
            # PSUM bank reuse >= 8 passes behind its reader so the PE never
            # stalls):
            #   rexp(4g+1) u(4g)   d(4g-2)
            #   rexp(4g+2) u(4g+1) d(4g-1)
            #   n2+sqrt-chain(g+1)
            #   rexp(4g+3) u(4g+2) d(4g)
            #   rexp(4g+4) u(4g+3) d(4g+1)
            def _rexp(c):
                if c < N_CHUNKS:
                    emit_rexp_vu(c)

            for g in range(N_GROUPS):
                c0 = 4 * g
                emit_x(c0 + 4)
                emit_x(c0 + 6)
                for j in range(4):
                    emit_sq(c0 + 4 + j)
                _rexp(c0 + 1)
                emit_u_phase(c0 + 0)
                if g > 0:
                    emit_d(c0 - 2)
                _rexp(c0 + 2)
                emit_u_phase(c0 + 1)
                if g > 0:
                    emit_d(c0 - 1)
                if g + 1 < N_GROUPS:
                    emit_norm_chain(g + 1)
                _rexp(c0 + 3)
                emit_u_phase(c0 + 2)
                emit_d(c0 + 0)
                _rexp(c0 + 4)
                emit_u_phase(c0 + 3)
                emit_d(c0 + 1)

            emit_d(N_CHUNKS - 2)
            emit_d(N_CHUNKS - 1)

    nc.compile()
    return nc


def _get_nc():
    if "nc" not in _CACHE:
        _CACHE["nc"] = _build_nc()
    return _CACHE["nc"]


def _make_in_map(shard, stat):
    """shard [n<=PADDED, 120] f32 -> hT [88, PADDED] f16 (vec+tens comps)."""
    n = shard.shape[0]
    buf = np.ones((NRIN, PADDED), np.float16)
    buf[:, :n] = shard[:, 32:120].astype(np.float16).T
    return {"hT": buf, **stat}


def _stat_map():
    s1, e4, p_ks, r_ks = _stationaries()
    stat = {"S1": s1, "E4": e4}
    for k in range(4):
        stat[f"P{k}"] = p_ks[k]
        stat[f"R{k}"] = r_ks[k]
    return stat


def _dev_row_maps():
    """Reference vdots/tdots order -> device d-row index."""
    vmap = np.empty(120, np.int64)
    for p in range(120):
        vmap[p] = (p // 40) * 41 + (p % 40)
    tmap = np.empty(28, np.int64)
    for q in range(28):
        tmap[q] = q * 41 + 40 if q < 3 else 123 + (q - 3)
    return vmap, tmap


def _assemble(out_block, res_c, shard):
    """Fill out_block [n, 204] from device outputs + host passthrough."""
    n = out_block.shape[0]
    vmap, tmap = _dev_row_maps()
    dx, dy = res_c["dx"], res_c["dy"]  # [128, PADDED] f16 each
    d = np.concatenate(
        [dx[0:41], dx[64:105], dy[0:41], dy[64:89]], axis=0
    )  # [148, PADDED] device pair order
    # n: [128, N_GROUPS*512] f32; chunk 4g+j strip at rows 32j:32j+24,
    # cols g*512:(g+1)*512 -> un-strip to [24, PADDED]
    ng = res_c["n"].reshape(4, 32, N_GROUPS, CHUNK)
    nn = ng.transpose(1, 2, 0, 3).reshape(32, PADDED)
    out_block[:, 0:32] = shard[:, 0:32]  # scalars passthrough
    out_block[:, 32:48] = nn[0:16, :n].T.astype(np.float32)
    out_block[:, 48:56] = nn[16:24, :n].T.astype(np.float32)
    out_block[:, 56:176] = d[vmap, :n].T.astype(np.float32)
    out_block[:, 176:204] = d[tmap, :n].T.astype(np.float32)


def _run_pjrt(nc, in_maps):
    """Execute the Bass module on N_CORES devices via PJRT/shard_map with
    per-device buffer assembly and per-shard fetch (avoids giant host
    concats, which trip transfer limits on the axon path)."""
    import jax
    from jax.sharding import Mesh, NamedSharding, PartitionSpec
    from jax.experimental.shard_map import shard_map
    from concourse import mybir
    from concourse.bass2jax import (
        _bass_exec_p,
        install_neuronx_cc_hook,
        partition_id_tensor,
    )

    install_neuronx_cc_hook()
    partition_name = nc.partition_id_tensor.name if nc.partition_id_tensor else None
    in_names, out_names, out_avals = [], [], []
    for alloc in nc.m.functions[0].allocations:
        if not isinstance(alloc, mybir.MemoryLocationSet):
            continue
        name = alloc.memorylocations[0].name
        if alloc.kind == "ExternalInput":
            if name != partition_name:
                in_names.append(name)
        elif alloc.kind == "ExternalOutput":
            out_names.append(name)
            shape = tuple(alloc.tensor_shape)
            dtype = mybir.dt.np(alloc.dtype)
            out_avals.append(jax.core.ShapedArray(shape, dtype))
    n_params = len(in_names)
    n_outs = len(out_avals)
    all_in_names = list(in_names) + out_names
    if partition_name is not None:
        all_in_names.append(partition_name)
    donate = tuple(range(n_params, n_params + n_outs))

    def _body(*args):
        operands = list(args)
        if partition_name is not None:
            operands.append(partition_id_tensor())
        outs = _bass_exec_p.bind(
            *operands,
            out_avals=tuple(out_avals),
            in_names=tuple(all_in_names),
            out_names=tuple(out_names),
            lowering_input_output_aliases=(),
            sim_require_finite=True,
            sim_require_nnan=True,
            nc=nc,
        )
        return tuple(outs)

    devices = jax.devices()[:N_CORES]
    mesh = Mesh(np.asarray(devices), ("core",))
    sharding = NamedSharding(mesh, PartitionSpec("core"))
    fn = jax.jit(
        shard_map(
            _body,
            mesh=mesh,
            in_specs=(PartitionSpec("core"),) * (n_params + n_outs),
            out_specs=(PartitionSpec("core"),) * n_outs,
            check_rep=False,
        ),
        donate_argnums=donate,
        keep_unused=True,
    )

    def make_global(per_core_arrays):
        a0 = per_core_arrays[0]
        gshape = (N_CORES * a0.shape[0],) + a0.shape[1:]
        bufs = [
            jax.device_put(per_core_arrays[c], devices[c]) for c in range(N_CORES)
        ]
        return jax.make_array_from_single_device_arrays(gshape, sharding, bufs)

    g_ins = [
        make_global([np.asarray(in_maps[c][nm]) for c in range(N_CORES)])
        for nm in in_names
    ]
    g_zeros = [
        make_global([np.zeros(av.shape, av.dtype) for _ in range(N_CORES)])
        for av in out_avals
    ]
    outs = fn(*g_ins, *g_zeros)
    jax.block_until_ready(outs)

    results = [dict() for _ in range(N_CORES)]
    for i, nm in enumerate(out_names):
        shards = sorted(
            outs[i].addressable_shards, key=lambda s: devices.index(s.device)
        )
        for c, sh in enumerate(shards):
            results[c][nm] = np.asarray(sh.data)
    return results


def kernel(h):
    h = np.asarray(h, dtype=np.float32)
    assert h.shape == (N_ATOMS, 120)

    nc = _get_nc()
    stat = _stat_map()
    in_maps = [
        _make_in_map(h[c * PER_CORE : (c + 1) * PER_CORE], stat)
        for c in range(N_CORES)
    ]
    res = _run_pjrt(nc, in_maps)

    out = np.empty((N_ATOMS, NOUT), np.float32)
    for c in range(N_CORES):
        _assemble(
            out[c * PER_CORE : (c + 1) * PER_CORE],
            res[c],
            h[c * PER_CORE : (c + 1) * PER_CORE],
        )
    return out
